# revision 1
# baseline (speedup 1.0000x reference)
"""Trainium2 Bass kernel for a 2-layer GAT + global mean pool + linear head.

Strategy (8 NeuronCores, SPMD single program, per-core data):
  - Nodes are partitioned into 8 contiguous shards of 1250; each core owns the
    edges whose dst falls in its shard (grouped by dst block of 128, sorted).
  - Dense phase (layer1) is replicated: every core computes h1 = x @ W1 for all
    10000 nodes and writes a node-major "gather table" [N, 640] bf16 to its own
    HBM: cols 0:512 = h1, 512:516 = per-node src attention logits (folded into
    the matmul via A1 = einsum(W1, att_src1) host-side precompute).
  - Edge phase: per dst-block, edge source rows are fetched with dma_gather
    (SWDGE indirect DMA), attention weights computed densely on-partition
    (edges on partitions), and segment-softmax + scatter-add are done as small
    dense matmuls against host-built one-hot segment matrices accumulated in
    PSUM.  Softmax runs without max-subtraction (logits are O(10)) and
    normalization is applied after aggregation: out = (sum p_e * h_src) / sum p_e.
  - Layer 2 input is exchanged with an 8-rank AllGather; pooled partials with
    an AllReduce.  Output [16, 10] f32 is identical on every core.
"""
import os
import sys
import numpy as np

for _p in ("/opt/trn_rl_repo", "/root/.axon_site/_ro/trn_rl_repo"):
    if os.path.isdir(_p) and _p not in sys.path:
        sys.path.append(_p)

import ml_dtypes

BF16 = ml_dtypes.bfloat16

# -------- problem constants (hardcoded per contest rules) --------
N = 10000
E = 160000
F_IN = 768
H1 = 4
C = 128
OUT = 10
G = 16
NEG_SLOPE = 0.2
P = 128
N_CORES = 8
KC1 = F_IN // P          # 6 k-chunks for layer-1 matmul
KC2 = (H1 * C) // P      # 4 k-chunks for layer-2 matmul
NCHUNK = (N + P - 1) // P   # 79 node chunks (last has 16 rows)
ROW1 = 640               # h-pack row: 512 h | 4 als | 124 pad   (1280 B bf16)
ROW2 = 256               # h2-pack row: 128 h2 | 1 als2 | 127 pad (512 B)
ROWA = 128               # ald row: 4 vals | 124 pad              (256 B)


def _bf(x):
    return np.ascontiguousarray(np.asarray(x, dtype=np.float32).astype(BF16))


def _prep(x, edge_index, batch, W1, att_src1, att_dst1, b1, W2, att_src2,
          att_dst2, b2, Wc, bc, n_cores=N_CORES):
    """Host-side index/layout preprocessing. Returns (common, per_core, meta)."""
    x = np.asarray(x, np.float32)
    edge_index = np.asarray(edge_index, np.int64)
    batch = np.asarray(batch, np.int64)
    nloc = N // n_cores
    nblk = (nloc + P - 1) // P
    src = np.concatenate([edge_index[0], np.arange(N, dtype=np.int64)])
    dst = np.concatenate([edge_index[1], np.arange(N, dtype=np.int64)])

    W1 = np.asarray(W1, np.float32)
    W2 = np.asarray(W2, np.float32)
    W1r = W1.reshape(F_IN, H1, C)
    A1 = np.concatenate([
        np.einsum('khc,hc->kh', W1r, np.asarray(att_src1, np.float32)),
        np.einsum('khc,hc->kh', W1r, np.asarray(att_dst1, np.float32)),
    ], axis=1)                                  # [768, 8]
    A2 = np.stack([W2 @ np.asarray(att_src2, np.float32)[0],
                   W2 @ np.asarray(att_dst2, np.float32)[0]], axis=1)  # [512, 2]

    cnt = np.bincount(batch, minlength=G).astype(np.float32)
    inv_cnt = 1.0 / np.maximum(cnt, 1.0)

    # per-core edge grouping by dst block
    core_blocks = []
    t_max = 1
    for c in range(n_cores):
        lo = c * nloc
        m = (dst >= lo) & (dst < lo + nloc)
        s_c, d_c = src[m], dst[m] - lo
        order = np.argsort(d_c, kind='stable')
        s_c, d_c = s_c[order], d_c[order]
        blocks = []
        for b in range(nblk):
            bm = (d_c >= b * P) & (d_c < min((b + 1) * P, nloc))
            blocks.append((s_c[bm], d_c[bm] - b * P))
            t_max = max(t_max, (len(blocks[-1][0]) + P - 1) // P)
        core_blocks.append(blocks)

    def idx_wrap(vals):
        v = np.asarray(vals, dtype=np.int16)
        out = np.zeros((16, len(v) // 16), dtype=np.int16)
        i = np.arange(len(v))
        out[i % 16, i // 16] = v
        return np.tile(out, (8, 1))

    common = dict(
        xT=_bf(x.T),
        w1=_bf(W1), a1=_bf(A1), w2=_bf(W2), a2=_bf(A2),
        wc=np.ascontiguousarray(np.asarray(Wc, np.float32)),
        bcb=np.ascontiguousarray(np.tile(np.asarray(bc, np.float32), (G, 1))),
        b1b=np.ascontiguousarray(np.tile(np.asarray(b1, np.float32), (P, 1))),
        b2b=np.ascontiguousarray(np.tile(np.asarray(b2, np.float32), (P, 1))),
    )

    per_core = []
    for c in range(n_cores):
        lo = c * nloc
        ne = t_max * P
        srccols, dstgcols, dstlcols = [], [], []
        seg = np.zeros((nblk * t_max, P, P), dtype=BF16)
        for b in range(nblk):
            sb, db = core_blocks[c][b]
            s_pad = np.zeros(ne, dtype=np.int64); s_pad[:len(sb)] = sb
            dg_pad = np.zeros(ne, dtype=np.int64); dg_pad[:len(db)] = db + b * P + lo
            dl_pad = np.zeros(ne, dtype=np.int64); dl_pad[:len(db)] = db + b * P
            srccols.append(idx_wrap(s_pad))
            dstgcols.append(idx_wrap(dg_pad))   # global dst id (layer-1 ald table)
            dstlcols.append(idx_wrap(dl_pad))   # local dst id (layer-2 ald table)
            ei = np.arange(len(db))
            seg[b * t_max + ei // P, ei % P, db] = BF16(1.0)
        poolm = np.zeros((nblk * P, G), dtype=np.float32)
        gg = batch[lo:lo + nloc]
        poolm[np.arange(nloc), gg] = inv_cnt[gg]
        per_core.append(dict(
            srcidx=np.ascontiguousarray(np.concatenate(srccols, axis=1)),
            dstgidx=np.ascontiguousarray(np.concatenate(dstgcols, axis=1)),
            dstlidx=np.ascontiguousarray(np.concatenate(dstlcols, axis=1)),
            # seg shipped pre-transposed: [128 (edge), T_tot * 128 (tile, dst)]
            seg=np.ascontiguousarray(seg.transpose(1, 0, 2).reshape(P, nblk * t_max * P)),
            # poolm shipped pre-transposed: [128 (node-in-chunk), nblk * 16]
            poolm=np.ascontiguousarray(
                poolm.reshape(nblk, P, G).transpose(1, 0, 2).reshape(P, nblk * G)),
        ))
    meta = dict(n_cores=n_cores, nloc=nloc, nblk=nblk, t_max=t_max)
    return common, per_core, meta


# ------------------------------------------------------------------
#  device program
# ------------------------------------------------------------------

def _build(meta, phases='full'):
    from concourse import bass, bacc, tile, mybir
    from concourse.masks import make_identity

    n_cores, nloc, nblk, t_max = (meta['n_cores'], meta['nloc'],
                                  meta['nblk'], meta['t_max'])
    ttot = nblk * t_max
    ncols = ttot * 8                      # idx tensor cols
    bf16, f32, i16 = mybir.dt.bfloat16, mybir.dt.float32, mybir.dt.int16

    nc = bacc.Bacc("TRN2", target_bir_lowering=False, debug=False,
                   num_devices=n_cores)

    # ---- I/O ----
    d_xT = nc.dram_tensor("xT", [F_IN, N], bf16, kind="ExternalInput")
    d_w1 = nc.dram_tensor("w1", [F_IN, 512], bf16, kind="ExternalInput")
    d_a1 = nc.dram_tensor("a1", [F_IN, 8], bf16, kind="ExternalInput")
    d_w2 = nc.dram_tensor("w2", [512, C], bf16, kind="ExternalInput")
    d_a2 = nc.dram_tensor("a2", [512, 2], bf16, kind="ExternalInput")
    d_wc = nc.dram_tensor("wc", [C, OUT], f32, kind="ExternalInput")
    d_bcb = nc.dram_tensor("bcb", [G, OUT], f32, kind="ExternalInput")
    d_b1b = nc.dram_tensor("b1b", [P, 512], f32, kind="ExternalInput")
    d_b2b = nc.dram_tensor("b2b", [P, C], f32, kind="ExternalInput")
    d_srci = nc.dram_tensor("srcidx", [P, ncols], i16, kind="ExternalInput")
    d_dgi = nc.dram_tensor("dstgidx", [P, ncols], i16, kind="ExternalInput")
    d_dli = nc.dram_tensor("dstlidx", [P, ncols], i16, kind="ExternalInput")
    d_seg = nc.dram_tensor("seg", [P, ttot * P], bf16, kind="ExternalInput")
    d_poolm = nc.dram_tensor("poolm", [P, nblk * G], f32, kind="ExternalInput")
    d_out = nc.dram_tensor("out", [G, OUT], f32, kind="ExternalOutput")

    with tile.TileContext(nc) as tc:
        with tc.tile_pool(name="dram", bufs=1, space="DRAM") as dram, \
             tc.tile_pool(name="const", bufs=1) as const, \
             tc.tile_pool(name="persist", bufs=1) as persist, \
             tc.tile_pool(name="psU", bufs=2, space="PSUM") as psU, \
             tc.tile_pool(name="psD", bufs=2, space="PSUM") as psD, \
             tc.tile_pool(name="psT", bufs=2, space="PSUM") as psT, \
             tc.tile_pool(name="psP", bufs=1, space="PSUM") as psP, \
             tc.tile_pool(name="work", bufs=2) as work, \
             tc.tile_pool(name="rows", bufs=3) as rows:

            # ---- DRAM internals ----
            h_pack = dram.tile([N, ROW1], bf16)          # layer-1 gather table
            ald_tab = dram.tile([N, ROWA], bf16)         # layer-1 dst-logit table
            h2_in = dram.tile([nloc, ROW2], bf16)        # AllGather send shard
            h2_full = dram.tile([N, ROW2], bf16)         # layer-2 gather table
            ald2_tab = dram.tile([nloc, ROWA], bf16)
            pool_in = dram.tile([P, G], f32)
            pool_out = dram.tile([P, G], f32)

            # ---- resident SBUF constants ----
            w1_sb = const.tile([P, KC1, 512], bf16)
            nc.sync.dma_start(out=w1_sb[:], in_=d_w1.ap().rearrange("(kc p) n -> p kc n", p=P))
            a1_sb = const.tile([P, KC1, 8], bf16)
            nc.sync.dma_start(out=a1_sb[:], in_=d_a1.ap().rearrange("(kc p) n -> p kc n", p=P))
            w2_sb = const.tile([P, KC2, C], bf16)
            nc.sync.dma_start(out=w2_sb[:], in_=d_w2.ap().rearrange("(kc p) n -> p kc n", p=P))
            a2_sb = const.tile([P, KC2, 2], bf16)
            nc.sync.dma_start(out=a2_sb[:], in_=d_a2.ap().rearrange("(kc p) n -> p kc n", p=P))
            wc_sb = const.tile([P, OUT], f32)
            nc.sync.dma_start(out=wc_sb[:], in_=d_wc[:, :])
            bcb_sb = const.tile([G, OUT], f32)
            nc.sync.dma_start(out=bcb_sb[:], in_=d_bcb[:, :])
            b1b_sb = const.tile([P, 512], f32)
            nc.sync.dma_start(out=b1b_sb[:], in_=d_b1b[:, :])
            b2b_sb = const.tile([P, C], f32)
            nc.sync.dma_start(out=b2b_sb[:], in_=d_b2b[:, :])
            srci_sb = const.tile([P, ncols], i16)
            nc.sync.dma_start(out=srci_sb[:], in_=d_srci[:, :])
            dgi_sb = const.tile([P, ncols], i16)
            nc.sync.dma_start(out=dgi_sb[:], in_=d_dgi[:, :])
            dli_sb = const.tile([P, ncols], i16)
            nc.sync.dma_start(out=dli_sb[:], in_=d_dli[:, :])
            seg_sb = const.tile([P, ttot, P], bf16)
            nc.sync.dma_start(out=seg_sb[:], in_=d_seg.ap())
            poolm_sb = const.tile([P, nblk, G], f32)
            nc.sync.dma_start(out=poolm_sb[:], in_=d_poolm.ap())
            ident = const.tile([P, P], bf16)
            make_identity(nc, ident[:])

            h1T_sb = persist.tile([P, KC2, nloc], bf16)   # relu(out1) transposed

            do_e1 = phases in ('d1e1', 'd1e1d2', 'nocoll', 'full')
            do_d2 = phases in ('d1e1d2', 'nocoll', 'full')
            do_e2 = phases in ('nocoll', 'full')
            do_coll = phases == 'full'

            # ================= D1: h1 = x @ W1 (all nodes, replicated) ======
            GRP = 10                                     # node chunks per x load
            with tc.tile_pool(name="xstage", bufs=2) as xstage:
                for j in range(NCHUNK):
                    g, jj = divmod(j, GRP)
                    if jj == 0:
                        ncols_g = min(GRP * P, N - g * GRP * P)
                        xg = xstage.tile([P, KC1, GRP * P], bf16, tag="xg")
                        nc.sync.dma_start(
                            out=xg[:, :, 0:ncols_g],
                            in_=d_xT.ap().rearrange("(kc p) n -> p kc n", p=P)[
                                :, :, g * GRP * P: g * GRP * P + ncols_g])
                    nd = min(P, N - j * P)
                    ph = psU.tile([P, 512], f32, tag="U")
                    pal = psD.tile([P, 8], f32, tag="den")
                    for kc in range(KC1):
                        lhs = xg[:, kc, jj * P: jj * P + nd]
                        nc.tensor.matmul(out=ph[0:nd, :], lhsT=lhs,
                                         rhs=w1_sb[:, kc, :],
                                         start=(kc == 0), stop=(kc == KC1 - 1))
                        nc.tensor.matmul(out=pal[0:nd, :], lhsT=lhs,
                                         rhs=a1_sb[:, kc, :],
                                         start=(kc == 0), stop=(kc == KC1 - 1))
                    hrow = rows.tile([P, ROW1], bf16, tag="hrow")
                    nc.gpsimd.memset(hrow[0:nd, 516:ROW1], 0.0)
                    nc.vector.tensor_copy(out=hrow[0:nd, 0:512], in_=ph[0:nd, :])
                    nc.vector.tensor_copy(out=hrow[0:nd, 512:516], in_=pal[0:nd, 0:4])
                    nc.sync.dma_start(out=h_pack[j * P: j * P + nd, :],
                                      in_=hrow[0:nd, :])
                    arow = rows.tile([P, ROWA], bf16, tag="arow")
                    nc.gpsimd.memset(arow[0:nd, 4:ROWA], 0.0)
                    nc.vector.tensor_copy(out=arow[0:nd, 0:4], in_=pal[0:nd, 4:8])
                    nc.sync.dma_start(out=ald_tab[j * P: j * P + nd, :],
                                      in_=arow[0:nd, :])

            # ================= E1: layer-1 edge phase (local dst blocks) ====
            nidx = t_max * P
            for b in range(nblk if do_e1 else 0):
                nd = min(P, nloc - b * P)
                cb = b * t_max * 8
                hg = work.tile([P, t_max, ROW1], bf16, tag="hg")
                ag = work.tile([P, t_max, ROWA], bf16, tag="ag")
                nc.gpsimd.dma_gather(hg[:], h_pack[:, :], srci_sb[:, cb:cb + t_max * 8],
                                     nidx, nidx, ROW1, single_packet=False)
                nc.gpsimd.dma_gather(ag[:], ald_tab[:, :], dgi_sb[:, cb:cb + t_max * 8],
                                     nidx, nidx, ROWA, single_packet=False)

                s32 = work.tile([P, t_max, 4], f32, tag="s32")
                nc.vector.tensor_tensor(out=s32[:], in0=hg[:, :, 512:516],
                                        in1=ag[:, :, 0:4], op=mybir.AluOpType.add)
                sa = work.tile([P, t_max, 4], f32, tag="sa")
                nc.vector.tensor_scalar_mul(sa[:], s32[:], NEG_SLOPE)
                e32 = work.tile([P, t_max, 4], f32, tag="e32")
                nc.vector.tensor_tensor(out=e32[:], in0=s32[:], in1=sa[:],
                                        op=mybir.AluOpType.max)
                pbf = work.tile([P, t_max, 4], bf16, tag="pbf")
                nc.scalar.activation(pbf[:], e32[:], mybir.ActivationFunctionType.Exp)

                for h in range(H1):
                    nc.vector.tensor_tensor(
                        out=hg[:, :, h * C:(h + 1) * C],
                        in0=hg[:, :, h * C:(h + 1) * C],
                        in1=pbf[:, :, h:h + 1].to_broadcast([P, t_max, C]),
                        op=mybir.AluOpType.mult)

                U = psU.tile([P, 512], f32, tag="U")
                for t in range(t_max):
                    nc.tensor.matmul(out=U[:, :], lhsT=seg_sb[:, b * t_max + t, :],
                                     rhs=hg[:, t, 0:512],
                                     start=(t == 0), stop=(t == t_max - 1))
                den = psD.tile([P, 4], f32, tag="den")
                for t in range(t_max):
                    nc.tensor.matmul(out=den[:, :], lhsT=seg_sb[:, b * t_max + t, :],
                                     rhs=pbf[:, t, :],
                                     start=(t == 0), stop=(t == t_max - 1))
                rec = work.tile([P, 4], f32, tag="rec")
                nc.vector.reciprocal(rec[0:nd, :], den[0:nd, :])
                o1 = work.tile([P, 512], f32, tag="o1")
                for h in range(H1):
                    nc.vector.tensor_scalar_mul(o1[0:nd, h * C:(h + 1) * C],
                                                U[0:nd, h * C:(h + 1) * C],
                                                rec[0:nd, h:h + 1])
                nc.vector.tensor_tensor(out=o1[0:nd, :], in0=o1[0:nd, :],
                                        in1=b1b_sb[0:nd, :], op=mybir.AluOpType.add)
                o1b = work.tile([P, 512], bf16, tag="o1b")
                nc.scalar.activation(o1b[0:nd, :], o1[0:nd, :],
                                     mybir.ActivationFunctionType.Relu)
                for kc in range(KC2):
                    tp = psT.tile([P, P], bf16, tag="tp")
                    nc.tensor.transpose(out=tp[:, 0:nd],
                                        in_=o1b[0:nd, kc * P:(kc + 1) * P],
                                        identity=ident[0:nd, 0:nd])
                    nc.vector.tensor_copy(out=h1T_sb[:, kc, b * P: b * P + nd],
                                          in_=tp[:, 0:nd])

            # ================= D2: h2 = relu(out1) @ W2 (local nodes) =======
            for j in range(nblk if do_d2 else 0):
                nd = min(P, nloc - j * P)
                p2 = psU.tile([P, C], f32, tag="U")
                p2a = psD.tile([P, 2], f32, tag="den")
                for kc in range(KC2):
                    lhs = h1T_sb[:, kc, j * P: j * P + nd]
                    nc.tensor.matmul(out=p2[0:nd, :], lhsT=lhs, rhs=w2_sb[:, kc, :],
                                     start=(kc == 0), stop=(kc == KC2 - 1))
                    nc.tensor.matmul(out=p2a[0:nd, :], lhsT=lhs, rhs=a2_sb[:, kc, :],
                                     start=(kc == 0), stop=(kc == KC2 - 1))
                r2 = rows.tile([P, ROW2], bf16, tag="r2")
                nc.gpsimd.memset(r2[0:nd, C + 1:ROW2], 0.0)
                nc.vector.tensor_copy(out=r2[0:nd, 0:C], in_=p2[0:nd, :])
                nc.vector.tensor_copy(out=r2[0:nd, C:C + 1], in_=p2a[0:nd, 0:1])
                nc.sync.dma_start(out=h2_in[j * P: j * P + nd, :], in_=r2[0:nd, :])
                a2row = rows.tile([P, ROWA], bf16, tag="a2row")
                nc.gpsimd.memset(a2row[0:nd, 1:ROWA], 0.0)
                nc.vector.tensor_copy(out=a2row[0:nd, 0:1], in_=p2a[0:nd, 1:2])
                nc.sync.dma_start(out=ald2_tab[j * P: j * P + nd, :],
                                  in_=a2row[0:nd, :])

            # ---- exchange layer-2 inputs ----
            if do_coll:
                nc.gpsimd.collective_compute(
                    "AllGather", mybir.AluOpType.bypass,
                    replica_groups=[list(range(n_cores))],
                    ins=[h2_in.opt()], outs=[h2_full.opt()])
            elif do_e2:
                nc.sync.dma_start(out=h2_full[0:nloc, :], in_=h2_in[:, :])

            # ================= E2: layer-2 edge phase =======================
            poolT = psP.tile([P, G], f32, tag="poolT")
            for b in range(nblk if do_e2 else 0):
                nd = min(P, nloc - b * P)
                cb = b * t_max * 8
                hg2 = work.tile([P, t_max, ROW2], bf16, tag="hg2")
                ag2 = work.tile([P, t_max, ROWA], bf16, tag="ag2")
                nc.gpsimd.dma_gather(hg2[:], h2_full[:, :], srci_sb[:, cb:cb + t_max * 8],
                                     nidx, nidx, ROW2, single_packet=False)
                nc.gpsimd.dma_gather(ag2[:], ald2_tab[:, :], dli_sb[:, cb:cb + t_max * 8],
                                     nidx, nidx, ROWA, single_packet=False)

                s2 = work.tile([P, t_max, 1], f32, tag="s2")
                nc.vector.tensor_tensor(out=s2[:], in0=hg2[:, :, C:C + 1],
                                        in1=ag2[:, :, 0:1], op=mybir.AluOpType.add)
                sa2 = work.tile([P, t_max, 1], f32, tag="sa2")
                nc.vector.tensor_scalar_mul(sa2[:], s2[:], NEG_SLOPE)
                e2 = work.tile([P, t_max, 1], f32, tag="e2")
                nc.vector.tensor_tensor(out=e2[:], in0=s2[:], in1=sa2[:],
                                        op=mybir.AluOpType.max)
                pbf2 = work.tile([P, t_max, 1], bf16, tag="pbf2")
                nc.scalar.activation(pbf2[:], e2[:], mybir.ActivationFunctionType.Exp)

                nc.vector.tensor_tensor(
                    out=hg2[:, :, 0:C], in0=hg2[:, :, 0:C],
                    in1=pbf2[:, :, 0:1].to_broadcast([P, t_max, C]),
                    op=mybir.AluOpType.mult)

                U2 = psU.tile([P, C], f32, tag="U")
                for t in range(t_max):
                    nc.tensor.matmul(out=U2[:, :], lhsT=seg_sb[:, b * t_max + t, :],
                                     rhs=hg2[:, t, 0:C],
                                     start=(t == 0), stop=(t == t_max - 1))
                den2 = psD.tile([P, 1], f32, tag="den")
                for t in range(t_max):
                    nc.tensor.matmul(out=den2[:, :], lhsT=seg_sb[:, b * t_max + t, :],
                                     rhs=pbf2[:, t, :],
                                     start=(t == 0), stop=(t == t_max - 1))
                rec2 = work.tile([P, 1], f32, tag="rec2")
                nc.vector.reciprocal(rec2[0:nd, :], den2[0:nd, :])
                o2 = work.tile([P, C], f32, tag="o2")
                nc.vector.tensor_scalar_mul(o2[0:nd, :], U2[0:nd, :], rec2[0:nd, 0:1])
                nc.vector.tensor_tensor(out=o2[0:nd, :], in0=o2[0:nd, :],
                                        in1=b2b_sb[0:nd, :], op=mybir.AluOpType.add)
                o2r = work.tile([P, C], f32, tag="o2r")
                nc.scalar.activation(o2r[0:nd, :], o2[0:nd, :],
                                     mybir.ActivationFunctionType.Relu)
                nc.tensor.matmul(out=poolT[:, :], lhsT=o2r[0:nd, :],
                                 rhs=poolm_sb[0:nd, b, :],
                                 start=(b == 0), stop=(b == nblk - 1))

            # ================= tail: pool exchange + classifier =============
            if not do_e2:
                dummy = work.tile([G, OUT], f32, tag="dummy")
                nc.vector.tensor_copy(out=dummy[:], in_=bcb_sb[:])
                nc.sync.dma_start(out=d_out[:, :], in_=dummy[:])
            else:
              poolT_sb = work.tile([P, G], f32, tag="poolT_sb")
              nc.vector.tensor_copy(out=poolT_sb[:], in_=poolT[:, :])
              nc.sync.dma_start(out=pool_in[:, :], in_=poolT_sb[:])
              if do_coll:
                  nc.gpsimd.collective_compute(
                      "AllReduce", mybir.AluOpType.add,
                      replica_groups=[list(range(n_cores))],
                      ins=[pool_in.opt()], outs=[pool_out.opt()])
              else:
                  nc.sync.dma_start(out=pool_out[:, :], in_=pool_in[:, :])
              poolF_sb = work.tile([P, G], f32, tag="poolF_sb")
              nc.sync.dma_start(out=poolF_sb[:], in_=pool_out[:, :])
              ofin = psD.tile([G, OUT], f32, tag="den")
              nc.tensor.matmul(out=ofin[:, :], lhsT=poolF_sb[:], rhs=wc_sb[:],
                               start=True, stop=True)
              ofin_sb = work.tile([G, OUT], f32, tag="ofin_sb")
              nc.vector.tensor_tensor(out=ofin_sb[:], in0=ofin[:, :], in1=bcb_sb[:],
                                      op=mybir.AluOpType.add)
              nc.sync.dma_start(out=d_out[:, :], in_=ofin_sb[:])

    nc.compile()
    return nc


# ------------------------------------------------------------------
#  runner
# ------------------------------------------------------------------

_CACHE = {}


def _get_nc(meta):
    key = (meta['n_cores'], meta['nblk'], meta['t_max'], meta['nloc'])
    if key not in _CACHE:
        _CACHE[key] = _build(meta)
    return _CACHE[key]


def _in_maps(common, per_core):
    maps = []
    for pc in per_core:
        m = dict(common)
        m['srcidx'] = pc['srcidx']
        m['dstgidx'] = pc['dstgidx']
        m['dstlidx'] = pc['dstlidx']
        m['seg'] = pc['seg']
        m['poolm'] = pc['poolm']
        maps.append(m)
    return maps


def kernel(**inputs) -> np.ndarray:
    common, per_core, meta = _prep(**inputs)
    nc = _get_nc(meta)
    from concourse.bass_utils import run_bass_kernel_spmd
    res = run_bass_kernel_spmd(nc, _in_maps(common, per_core),
                               core_ids=list(range(meta['n_cores'])))
    return np.asarray(res.results[0]['out'], np.float32).reshape(-1)



# revision 37
# speedup vs baseline: 3.9006x; 3.9006x over previous
"""Trainium2 Bass kernel for a 2-layer GAT + global mean pool + linear head.

Strategy (8 NeuronCores, SPMD single program, per-core data):
  - Nodes partitioned into 8 shards of 1250; each core owns edges whose dst
    falls in its shard (grouped by dst block of 128).
  - D1 (layer-1 dense matmul) is replicated: every core computes
    h1 = x @ W1 for all 10000 nodes and writes a node-major gather table,
    SPLIT into 4 quartile tables so edge gathers can start while D1 is
    still running (gather for quartile q only waits on that table).
  - Row layout [512 h | 4 al_src | 4 al_dst | pad] (640 cols bf16 = 1280 B,
    satisfying the 256 B dma_gather granularity).
  - Per-edge dst attention logits are NOT gathered (descriptor generation on
    GPSIMD at ~8 ns/idx made those gathers half the kernel): instead a
    host-built transposed one-hot (segT) is streamed per block and a PE
    matmul broadcasts the block's 128 local dst logits to edge lanes.
    Layer-1 local dst logits ship from the host (ald1 = x_local @ A1_dst);
    layer-2 ones are written from D2's PSUM (D2 blocks are local-aligned).
  - Segment softmax runs without max subtraction; normalization after
    aggregation: out = (sum p_e * h_src) / sum p_e.
  - D2 is interleaved with E1 per block; the layer-2 AllGather is split in
    two (local rows 0:640 after block 4, 640:1250 after block 9) so E2
    gathers from the first half overlap the second collective.
  - E2 aggregation gets its softmax denominator for free: per-edge p is
    written into gather-row column 128 and aggregated by the same matmul.
  - Pooled partials are combined with an AllReduce; output [16, 10] f32 is
    identical on every core.
"""
import os
import sys
import numpy as np

for _p in ("/opt/trn_rl_repo", "/root/.axon_site/_ro/trn_rl_repo"):
    if os.path.isdir(_p) and _p not in sys.path:
        sys.path.append(_p)

import ml_dtypes

BF16 = ml_dtypes.bfloat16

# -------- problem constants (hardcoded per contest rules) --------
N = 10000
E = 160000
F_IN = 768
H1 = 4
C = 128
OUT = 10
G = 16
NEG_SLOPE = 0.2
P = 128
N_CORES = 8
KC1 = F_IN // P          # 6 k-chunks for layer-1 matmul
KC2 = (H1 * C) // P      # 4 k-chunks for layer-2 matmul
NCHUNK = (N + P - 1) // P   # 79 node chunks (last has 16 rows)
ROW1 = 640               # h-pack row: 512 h | 4 als | 4 ald | 120 pad
ROW2 = 256               # h2-pack row bytes: 128 f8 h2 | bf16 als2 | bf16 ald2 | pad
NSPLIT1 = 4              # h-pack quartile tables (D1/E1 overlap)
SPL_ROWS = N // NSPLIT1  # 2500 rows per quartile table
NALOC = 640              # rows of the local shard in the "a" AllGather half
NBLOC = 610              # rows in the "b" half (1250 - 640)


def _bf(x):
    return np.ascontiguousarray(np.asarray(x, dtype=np.float32).astype(BF16))


def _idx_wrap(vals):
    v = np.asarray(vals, dtype=np.int16)
    out = np.zeros((16, len(v) // 16), dtype=np.int16)
    i = np.arange(len(v))
    out[i % 16, i // 16] = v
    return np.tile(out, (8, 1))


def _prep(x, edge_index, batch, W1, att_src1, att_dst1, b1, W2, att_src2,
          att_dst2, b2, Wc, bc, n_cores=N_CORES):
    """Host-side index/layout preprocessing. Returns (common, per_core, meta)."""
    x = np.asarray(x, np.float32)
    edge_index = np.asarray(edge_index, np.int64)
    batch = np.asarray(batch, np.int64)
    nloc = N // n_cores
    nblk = (nloc + P - 1) // P
    src = np.concatenate([edge_index[0], np.arange(N, dtype=np.int64)])
    dst = np.concatenate([edge_index[1], np.arange(N, dtype=np.int64)])

    W1 = np.asarray(W1, np.float32)
    W2 = np.asarray(W2, np.float32)
    W1r = W1.reshape(F_IN, H1, C)
    A1 = np.concatenate([
        np.einsum('khc,hc->kh', W1r, np.asarray(att_src1, np.float32)),
        np.einsum('khc,hc->kh', W1r, np.asarray(att_dst1, np.float32)),
    ], axis=1)                                  # [768, 8]
    A2 = np.stack([W2 @ np.asarray(att_src2, np.float32)[0],
                   W2 @ np.asarray(att_dst2, np.float32)[0]], axis=1)  # [512, 2]

    cnt = np.bincount(batch, minlength=G).astype(np.float32)
    inv_cnt = 1.0 / np.maximum(cnt, 1.0)

    # ---- per-core edge partition, blocked by dst, split by src range ----
    # E1 split: src quartile (h-pack table q).  E2 split: src in first 640
    # rows of its shard ("a" AllGather half) or the remaining 610 ("b").
    core_e1 = []   # [core][block][q] -> (src_local_in_table, )
    core_e2 = []   # [core][block][half] -> (row_in_half_table, )
    core_dst = []  # [core][block][q or half] -> dst_local_in_block
    s1_max = 1
    s2_max = 1
    for c in range(n_cores):
        lo = c * nloc
        m = (dst >= lo) & (dst < lo + nloc)
        s_c, d_c = src[m], dst[m] - lo
        blocks1, blocks2 = [], []
        for b in range(nblk):
            bm = (d_c >= b * P) & (d_c < min((b + 1) * P, nloc))
            sb, db = s_c[bm], d_c[bm] - b * P
            q = sb // SPL_ROWS
            g1 = [(sb[q == qq] - qq * SPL_ROWS, db[q == qq]) for qq in range(NSPLIT1)]
            blocks1.append(g1)
            s1_max = max(s1_max, max(len(gg[0]) for gg in g1))
            sc = sb // nloc                      # owning core of src
            r = sb - sc * nloc                   # row within shard
            isa = r < NALOC
            rowa = sc[isa] * NALOC + r[isa]
            rowb = sc[~isa] * NBLOC + (r[~isa] - NALOC)
            g2 = [(rowa, db[isa]), (rowb, db[~isa])]
            blocks2.append(g2)
            s2_max = max(s2_max, max(len(gg[0]) for gg in g2))
        core_e1.append(blocks1)
        core_e2.append(blocks2)

    # per-(block, split) padded sizes: max over cores only (SPMD needs the
    # same instruction shapes on every core, but they can vary per block)
    def _rup(v):
        return max(P, ((v + P - 1) // P) * P)
    sz1 = tuple(tuple(_rup(max(len(core_e1[c][b][q][0]) for c in range(n_cores)))
                      for q in range(NSPLIT1)) for b in range(nblk))
    sz2 = tuple(tuple(_rup(max(len(core_e2[c][b][hh][0]) for c in range(n_cores)))
                      for hh in range(2)) for b in range(nblk))
    tb1 = [sum(sz1[b]) // P for b in range(nblk)]
    tb2 = [sum(sz2[b]) // P for b in range(nblk)]
    ttot1 = sum(tb1)
    ttot2 = sum(tb2)

    F8 = ml_dtypes.float8_e4m3
    common = dict(
        xTf8=np.ascontiguousarray(x.T.astype(F8)),
        w1f8=np.ascontiguousarray(W1.astype(F8)),
        a1f8=np.ascontiguousarray(A1.astype(F8)), w2=_bf(W2), a2=_bf(A2),
        wc=np.ascontiguousarray(np.asarray(Wc, np.float32)),
        bcb=np.ascontiguousarray(np.tile(np.asarray(bc, np.float32), (G, 1))),
        b1b=np.ascontiguousarray(np.tile(np.asarray(b1, np.float32), (P, 1))),
        invcnt=np.ascontiguousarray(inv_cnt.reshape(G, 1)),
        b2b=np.ascontiguousarray(np.tile(np.asarray(b2, np.float32), (P, 1))),
    )

    tOff1 = np.concatenate([[0], np.cumsum(tb1)]).astype(int)
    tOff2 = np.concatenate([[0], np.cumsum(tb2)]).astype(int)
    per_core = []
    for c in range(n_cores):
        lo = c * nloc
        srccols, i2acols, i2bcols = [], [], []
        seg1 = np.zeros((ttot1, P, P), dtype=BF16)
        seg2 = np.zeros((ttot2, P, P), dtype=BF16)
        for b in range(nblk):
            boff = 0
            for q in range(NSPLIT1):
                sb, db = core_e1[c][b][q]
                s_pad = np.zeros(sz1[b][q], dtype=np.int64); s_pad[:len(sb)] = sb
                srccols.append(_idx_wrap(s_pad))
                ei = np.arange(len(db)) + boff
                seg1[tOff1[b] + ei // P, ei % P, db] = BF16(1.0)
                boff += sz1[b][q]
            boff = 0
            for hh in range(2):
                sb, db = core_e2[c][b][hh]
                s_pad = np.zeros(sz2[b][hh], dtype=np.int64); s_pad[:len(sb)] = sb
                (i2acols if hh == 0 else i2bcols).append(_idx_wrap(s_pad))
                ei = np.arange(len(db)) + boff
                seg2[tOff2[b] + ei // P, ei % P, db] = BF16(1.0)
                boff += sz2[b][hh]
        poolm = np.zeros((nblk * P, G), dtype=np.float32)
        gg = batch[lo:lo + nloc]
        poolm[np.arange(nloc), gg] = 1.0
        # host-computed layer-1 dst logits for the local shard: [P, nblk, 4]
        ald1 = np.zeros((P, nblk, 4), dtype=np.float32)
        av = x[lo:lo + nloc] @ A1[:, 4:8]
        for b in range(nblk):
            nd = min(P, nloc - b * P)
            ald1[0:nd, b, :] = av[b * P: b * P + nd]
        per_core.append(dict(
            srcidx=np.ascontiguousarray(np.concatenate(srccols, axis=1)),
            idx2a=np.ascontiguousarray(np.concatenate(i2acols, axis=1)),
            idx2b=np.ascontiguousarray(np.concatenate(i2bcols, axis=1)),
            # seg tables shipped pre-transposed per tile:
            #   seg*:  [128 (edge), T * 128 (tile, dst)]
            #   segT*: [128 (dst),  T * 128 (tile, edge)]
            seg1=np.ascontiguousarray(seg1.transpose(1, 0, 2).reshape(P, -1)),
            segT1=np.ascontiguousarray(seg1.transpose(2, 0, 1).reshape(P, -1)),
            seg2=np.ascontiguousarray(seg2.transpose(1, 0, 2).reshape(P, -1)),
            segT2=np.ascontiguousarray(seg2.transpose(2, 0, 1).reshape(P, -1)),
            poolm=np.ascontiguousarray(
                poolm.reshape(nblk, P, G).transpose(1, 0, 2)
                .reshape(P, nblk * G).astype(BF16)),
            ald1=np.ascontiguousarray(ald1.astype(BF16)),
        ))
    meta = dict(n_cores=n_cores, nloc=nloc, nblk=nblk, sz1=sz1, sz2=sz2)
    return common, per_core, meta


# ------------------------------------------------------------------
#  device program
# ------------------------------------------------------------------

def _build(meta):
    from concourse import bass, bacc, tile, mybir
    from concourse.masks import make_identity

    n_cores, nloc, nblk = meta['n_cores'], meta['nloc'], meta['nblk']
    sz1, sz2 = meta['sz1'], meta['sz2']
    tb1 = [sum(sz1[b]) // P for b in range(nblk)]
    tb2 = [sum(sz2[b]) // P for b in range(nblk)]
    ttot1 = sum(tb1)
    ttot2 = sum(tb2)
    tOff1 = [0] + list(np.cumsum(tb1).astype(int))
    tOff2 = [0] + list(np.cumsum(tb2).astype(int))
    tb1max = max(tb1)
    tb2max = max(tb2)
    ncol1 = sum(sum(s) for s in sz1) // 16     # total idx columns, layer 1
    ncol2a = sum(s[0] for s in sz2) // 16
    ncol2b = sum(s[1] for s in sz2) // 16
    cOff1 = [0] + list(np.cumsum([s // 16 for b in range(nblk)
                                  for s in sz1[b]]).astype(int))
    cOff2a = [0] + list(np.cumsum([sz2[b][0] // 16 for b in range(nblk)]).astype(int))
    cOff2b = [0] + list(np.cumsum([sz2[b][1] // 16 for b in range(nblk)]).astype(int))
    bf16, f32, i16 = mybir.dt.bfloat16, mybir.dt.float32, mybir.dt.int16
    f8 = mybir.dt.float8e4
    AF = mybir.ActivationFunctionType
    ALU = mybir.AluOpType
    DR = mybir.MatmulPerfMode.DoubleRow

    nc = bacc.Bacc("TRN2", target_bir_lowering=False, debug=False,
                   num_devices=n_cores, num_swdge_queues=4)

    # ---- I/O ----
    d_xTf8 = nc.dram_tensor("xTf8", [F_IN, N], f8, kind="ExternalInput")
    d_w1f8 = nc.dram_tensor("w1f8", [F_IN, 512], f8, kind="ExternalInput")
    d_a1 = nc.dram_tensor("a1f8", [F_IN, 8], f8, kind="ExternalInput")
    d_w2 = nc.dram_tensor("w2", [512, C], bf16, kind="ExternalInput")
    d_a2 = nc.dram_tensor("a2", [512, 2], bf16, kind="ExternalInput")
    d_wc = nc.dram_tensor("wc", [C, OUT], f32, kind="ExternalInput")
    d_bcb = nc.dram_tensor("bcb", [G, OUT], f32, kind="ExternalInput")
    d_b1b = nc.dram_tensor("b1b", [P, 512], f32, kind="ExternalInput")
    d_b2b = nc.dram_tensor("b2b", [P, C], f32, kind="ExternalInput")
    d_srci = nc.dram_tensor("srcidx", [P, ncol1], i16, kind="ExternalInput")
    d_i2a = nc.dram_tensor("idx2a", [P, ncol2a], i16, kind="ExternalInput")
    d_i2b = nc.dram_tensor("idx2b", [P, ncol2b], i16, kind="ExternalInput")
    d_seg1 = nc.dram_tensor("seg1", [P, ttot1 * P], bf16, kind="ExternalInput")
    d_segT1 = nc.dram_tensor("segT1", [P, ttot1 * P], bf16, kind="ExternalInput")
    d_seg2 = nc.dram_tensor("seg2", [P, ttot2 * P], bf16, kind="ExternalInput")
    d_segT2 = nc.dram_tensor("segT2", [P, ttot2 * P], bf16, kind="ExternalInput")
    d_poolm = nc.dram_tensor("poolm", [P, nblk * G], bf16, kind="ExternalInput")
    d_invc = nc.dram_tensor("invcnt", [G, 1], f32, kind="ExternalInput")
    d_ald1 = nc.dram_tensor("ald1", [P, nblk, 4], bf16, kind="ExternalInput")
    d_out = nc.dram_tensor("out", [G, OUT], f32, kind="ExternalOutput")

    with tile.TileContext(nc) as tc:
        with tc.tile_pool(name="dram", bufs=1, space="DRAM") as dram, \
             tc.tile_pool(name="const", bufs=1) as const, \
             tc.tile_pool(name="persist", bufs=1) as persist, \
             tc.tile_pool(name="psU", bufs=3, space="PSUM") as psU, \
             tc.tile_pool(name="psD", bufs=2, space="PSUM") as psD, \
             tc.tile_pool(name="psT", bufs=1, space="PSUM") as psT, \
             tc.tile_pool(name="psA", bufs=1, space="PSUM") as psA, \
             tc.tile_pool(name="psP", bufs=1, space="PSUM") as psP, \
             tc.tile_pool(name="segs", bufs=2) as segs, \
             tc.tile_pool(name="hgp", bufs=3) as hgp, \
             tc.tile_pool(name="hgp2", bufs=3) as hgp2, \
             tc.tile_pool(name="work", bufs=2) as work, \
             tc.tile_pool(name="rows", bufs=2) as rows:

            # ---- DRAM internals ----
            hp = [dram.tile([SPL_ROWS, ROW1], bf16, name=f"hp{q}", tag=f"hp{q}")
                  for q in range(NSPLIT1)]
            h2ina = dram.tile([NALOC, ROW2], f8)
            h2inb = dram.tile([NBLOC, ROW2], f8)
            h2fa = dram.tile([n_cores * NALOC, ROW2], f8, addr_space="Shared")
            h2fb = dram.tile([n_cores * NBLOC, ROW2], f8, addr_space="Shared")
            pool_in = dram.tile([P, G], f32)
            pool_out = dram.tile([P, G], f32)

            # ---- resident SBUF constants ----
            w1f8_sb = const.tile([P, KC1, 512], f8)
            nc.sync.dma_start(out=w1f8_sb[:], in_=d_w1f8.ap().rearrange("(kc p) n -> p kc n", p=P))
            a1_sb = const.tile([P, KC1, 8], f8)
            nc.sync.dma_start(out=a1_sb[:], in_=d_a1.ap().rearrange("(kc p) n -> p kc n", p=P))
            w2_sb = const.tile([P, KC2, C], bf16)
            nc.sync.dma_start(out=w2_sb[:], in_=d_w2.ap().rearrange("(kc p) n -> p kc n", p=P))
            a2_sb = const.tile([P, KC2, 2], bf16)
            nc.sync.dma_start(out=a2_sb[:], in_=d_a2.ap().rearrange("(kc p) n -> p kc n", p=P))
            wc_sb = const.tile([P, OUT], f32)
            nc.sync.dma_start(out=wc_sb[:], in_=d_wc[:, :])
            bcb_sb = const.tile([G, OUT], f32)
            nc.sync.dma_start(out=bcb_sb[:], in_=d_bcb[:, :])
            b1b_sb = const.tile([P, 512], f32)
            nc.sync.dma_start(out=b1b_sb[:], in_=d_b1b[:, :])
            b2b_sb = const.tile([P, C], f32)
            nc.sync.dma_start(out=b2b_sb[:], in_=d_b2b[:, :])
            srci_sb = const.tile([P, ncol1], i16)
            nc.sync.dma_start(out=srci_sb[:], in_=d_srci[:, :])
            i2a_sb = const.tile([P, ncol2a], i16)
            nc.sync.dma_start(out=i2a_sb[:], in_=d_i2a[:, :])
            i2b_sb = const.tile([P, ncol2b], i16)
            nc.sync.dma_start(out=i2b_sb[:], in_=d_i2b[:, :])
            poolm_sb = const.tile([P, nblk, G], bf16)
            nc.sync.dma_start(out=poolm_sb[:], in_=d_poolm.ap())
            invc_sb = const.tile([G, 1], f32)
            nc.sync.dma_start(out=invc_sb[:], in_=d_invc[:, :])
            ald1_sb = const.tile([P, nblk, 4], bf16)
            nc.sync.dma_start(out=ald1_sb[:], in_=d_ald1.ap())
            ident = const.tile([P, P], bf16)
            make_identity(nc, ident[:])

            # pre-zero the gather ring buffers once so negative-index
            # padding (skipped rows) never exposes uninitialized SBUF
            for _ in range(3):
                hgz = hgp.tile([P, tb1max, ROW1], bf16, tag="hg", name="hgz")
                nc.gpsimd.memset(hgz[:], 0.0)
                hg2z = hgp2.tile([P, tb2max, ROW2], f8, tag="hg2", name="hg2z")
                nc.gpsimd.memset(hg2z[:], 0.0)

            h1T_sb = persist.tile([P, KC2, nloc], bf16)   # relu(out1) transposed
            ald2_sb = persist.tile([P, nblk, 1], bf16)    # layer-2 dst logits

            # ================= D1: h1 = x @ W1 (all nodes, replicated) ======
            GRP = 8                                      # node chunks per x load
            PAIR = 2                                     # chunks per h-pack write
            with tc.tile_pool(name="xstage", bufs=2) as xstage:
                for j in range(NCHUNK):
                    g, jj = divmod(j, GRP)
                    if jj == 0:
                        ncols_g = min(GRP * P, N - g * GRP * P)
                        xgf8 = xstage.tile([P, KC1, GRP * P], f8, tag="xgf8")
                        nc.sync.dma_start(
                            out=xgf8[:, :, 0:ncols_g],
                            in_=d_xTf8.ap().rearrange("(kc p) n -> p kc n", p=P)[
                                :, :, g * GRP * P: g * GRP * P + ncols_g])
                    nd = min(P, N - j * P)
                    ph = psU.tile([P, 512], f32, tag="U")
                    pal = psD.tile([P, 8], f32, tag="den")
                    # fp8 DoubleRow: K=256 per instruction at 2x rate
                    for kc2 in range(KC1 // 2):
                        lhs = xgf8[:, 2 * kc2:2 * kc2 + 2, jj * P: jj * P + nd]
                        nc.tensor.matmul(out=ph[0:nd, :], lhsT=lhs,
                                         rhs=w1f8_sb[:, 2 * kc2:2 * kc2 + 2, :],
                                         start=(kc2 == 0), stop=(kc2 == KC1 // 2 - 1),
                                         perf_mode=DR)
                        nc.tensor.matmul(out=pal[0:nd, :], lhsT=lhs,
                                         rhs=a1_sb[:, 2 * kc2:2 * kc2 + 2, :],
                                         start=(kc2 == 0), stop=(kc2 == KC1 // 2 - 1),
                                         perf_mode=DR)
                    jp = j % PAIR
                    if jp == 0:
                        hrow = rows.tile([P, PAIR, ROW1], bf16, tag="hrow")
                    nc.vector.memset(hrow[0:nd, jp, 520:ROW1], 0.0)
                    nc.vector.tensor_copy(out=hrow[0:nd, jp, 0:512], in_=ph[0:nd, :])
                    nc.vector.tensor_copy(out=hrow[0:nd, jp, 512:520], in_=pal[0:nd, :])
                    # write into the quartile table(s); batches may straddle
                    if jp == PAIR - 1 or j == NCHUNK - 1:
                        j0 = j - jp
                        rlo, rhi = j0 * P, j * P + nd
                        if rlo // SPL_ROWS == (rhi - 1) // SPL_ROWS and jp == PAIR - 1:
                            q = rlo // SPL_ROWS
                            nc.scalar.dma_start(
                                out=hp[q][rlo - q * SPL_ROWS: rhi - q * SPL_ROWS, :]
                                .rearrange("(c p) r -> p c r", p=P),
                                in_=hrow[:, :, :])
                        else:
                            # straddling or short batch: per chunk-piece DMAs
                            for jc in range(j0, j + 1):
                                c0, c1 = jc * P, jc * P + min(P, N - jc * P)
                                r0 = c0
                                while r0 < c1:
                                    q = r0 // SPL_ROWS
                                    r1 = min(c1, (q + 1) * SPL_ROWS)
                                    nc.scalar.dma_start(
                                        out=hp[q][r0 - q * SPL_ROWS: r1 - q * SPL_ROWS, :],
                                        in_=hrow[r0 - c0: r1 - c0, jc - j0, :])
                                    r0 = r1

            # ============ E1 + D2 interleaved per dst block =================
            for b in range(nblk):
                nd = min(P, nloc - b * P)
                tb = tb1[b]
                hg = hgp.tile([P, tb1max, ROW1], bf16, tag="hg")
                ts = 0
                for q in range(NSPLIT1):
                    sz = sz1[b][q]
                    cb = cOff1[b * NSPLIT1 + q]
                    nc.gpsimd.dma_gather(
                        hg[:, ts:ts + sz // P, :], hp[q][:, :],
                        srci_sb[:, cb:cb + sz // 16], sz, sz, ROW1,
                        single_packet=False,
                        queue_num=1 + (b * NSPLIT1 + q) % 3)
                    ts += sz // P
                seg1b = segs.tile([P, tb1max, P], bf16, tag="seg")
                nc.sync.dma_start(
                    out=seg1b[:, 0:tb, :],
                    in_=d_seg1[:, tOff1[b] * P: tOff1[b + 1] * P]
                    .rearrange("p (t q) -> p t q", q=P))
                segT1b = segs.tile([P, tb1max, P], bf16, tag="segT")
                nc.sync.dma_start(
                    out=segT1b[:, 0:tb, :],
                    in_=d_segT1[:, tOff1[b] * P: tOff1[b + 1] * P]
                    .rearrange("p (t q) -> p t q", q=P))

                # broadcast local dst logits to edge lanes via one-hot^T
                pald = psA.tile([P, tb1max, 4], f32, tag="pald")
                for t in range(tb):
                    nc.tensor.matmul(out=pald[:, t, :], lhsT=segT1b[:, t, :],
                                     rhs=ald1_sb[:, b, :], start=True, stop=True)
                paldc = work.tile([P, tb1max, 4], bf16, tag="paldc")
                nc.vector.tensor_copy(out=paldc[:, 0:tb, :], in_=pald[:, 0:tb, :])

                s32 = work.tile([P, tb1max, 4], f32, tag="s32")
                nc.vector.tensor_tensor(out=s32[:, 0:tb, :],
                                        in0=hg[:, 0:tb, 512:516],
                                        in1=paldc[:, 0:tb, :], op=ALU.add)
                e32 = work.tile([P, tb1max, 4], f32, tag="e32")
                nc.vector.scalar_tensor_tensor(
                    out=e32[:, 0:tb, :], in0=s32[:, 0:tb, :], scalar=NEG_SLOPE,
                    in1=s32[:, 0:tb, :], op0=ALU.mult, op1=ALU.max)
                pbf = work.tile([P, tb1max, 4], bf16, tag="pbf")
                nc.scalar.activation(pbf[:, 0:tb, :], e32[:, 0:tb, :], AF.Exp)
                pbf32 = work.tile([P, tb1max, 4], f32, tag="pbf32")
                nc.scalar.activation(pbf32[:, 0:tb, :], e32[:, 0:tb, :], AF.Exp)

                for h in range(2):
                    nc.vector.tensor_tensor(
                        out=hg[:, 0:tb, h * C:(h + 1) * C],
                        in0=hg[:, 0:tb, h * C:(h + 1) * C],
                        in1=pbf[:, 0:tb, h:h + 1].to_broadcast([P, tb, C]),
                        op=ALU.mult)
                for h in range(2, H1):
                    for t in range(tb):
                        nc.scalar.activation(
                            hg[:, t, h * C:(h + 1) * C],
                            hg[:, t, h * C:(h + 1) * C],
                            AF.Copy, scale=pbf32[:, t, h:h + 1])

                U = psU.tile([P, 512], f32, tag="U")
                for t in range(tb):
                    nc.tensor.matmul(out=U[:, :], lhsT=seg1b[:, t, :],
                                     rhs=hg[:, t, 0:512],
                                     start=(t == 0), stop=(t == tb - 1))
                den = psD.tile([P, 4], f32, tag="den")
                for t in range(tb):
                    nc.tensor.matmul(out=den[:, :], lhsT=seg1b[:, t, :],
                                     rhs=pbf[:, t, :],
                                     start=(t == 0), stop=(t == tb - 1))
                rec = work.tile([P, 4], f32, tag="rec")
                nc.vector.reciprocal(rec[0:nd, :], den[0:nd, :])
                o1b = work.tile([P, 512], bf16, tag="o1b")
                for h in range(H1):
                    nc.vector.scalar_tensor_tensor(
                        out=o1b[0:nd, h * C:(h + 1) * C],
                        in0=U[0:nd, h * C:(h + 1) * C],
                        scalar=rec[0:nd, h:h + 1],
                        in1=b1b_sb[0:nd, h * C:(h + 1) * C],
                        op0=ALU.mult, op1=ALU.add)
                nc.scalar.activation(o1b[0:nd, :], o1b[0:nd, :], AF.Relu)
                for kc in range(KC2):
                    tp = psT.tile([P, P], bf16, tag="tp")
                    nc.tensor.transpose(out=tp[:, 0:nd],
                                        in_=o1b[0:nd, kc * P:(kc + 1) * P],
                                        identity=ident[0:nd, 0:nd])
                    nc.vector.tensor_copy(out=h1T_sb[:, kc, b * P: b * P + nd],
                                          in_=tp[:, 0:nd])

                # ---- D2 for this block (local rows b*P .. b*P+nd) ----
                p2 = psU.tile([P, C], f32, tag="U")
                p2a = psD.tile([P, 2], f32, tag="den")
                for kc in range(KC2):
                    lhs = h1T_sb[:, kc, b * P: b * P + nd]
                    nc.tensor.matmul(out=p2[0:nd, :], lhsT=lhs, rhs=w2_sb[:, kc, :],
                                     start=(kc == 0), stop=(kc == KC2 - 1))
                    nc.tensor.matmul(out=p2a[0:nd, :], lhsT=lhs, rhs=a2_sb[:, kc, :],
                                     start=(kc == 0), stop=(kc == KC2 - 1))
                r2 = rows.tile([P, ROW2], f8, tag="r2")
                nc.vector.memset(r2[0:nd, C + 4:ROW2], 0.0)
                nc.vector.tensor_copy(out=r2[0:nd, 0:C], in_=p2[0:nd, :])
                nc.vector.tensor_copy(out=r2[0:nd, C:C + 4].bitcast(bf16),
                                      in_=p2a[0:nd, :])
                nc.vector.tensor_copy(out=ald2_sb[0:nd, b, :], in_=p2a[0:nd, 1:2])
                if b < 5:
                    nc.sync.dma_start(out=h2ina[b * P: b * P + nd, :],
                                      in_=r2[0:nd, :])
                else:
                    r0 = b * P - NALOC
                    nc.sync.dma_start(out=h2inb[r0: r0 + nd, :], in_=r2[0:nd, :])
                if b == 4:
                    nc.gpsimd.collective_compute(
                        "AllGather", ALU.bypass,
                        replica_groups=[list(range(n_cores))],
                        ins=[h2ina.opt()], outs=[h2fa.opt()])
                if b == nblk - 1:
                    nc.gpsimd.collective_compute(
                        "AllGather", ALU.bypass,
                        replica_groups=[list(range(n_cores))],
                        ins=[h2inb.opt()], outs=[h2fb.opt()])

            # ================= E2: layer-2 edge phase =======================
            poolT = psP.tile([P, G], f32, tag="poolT")
            for b in range(nblk):
                nd = min(P, nloc - b * P)
                tb = tb2[b]
                sza, szb = sz2[b]
                hg2 = hgp2.tile([P, tb2max, ROW2], f8, tag="hg2")
                nc.gpsimd.dma_gather(
                    hg2[:, 0:sza // P, :], h2fa[:, :],
                    i2a_sb[:, cOff2a[b]: cOff2a[b] + sza // 16], sza, sza, ROW2,
                    single_packet=False, queue_num=1 + (2 * b) % 3)
                nc.gpsimd.dma_gather(
                    hg2[:, sza // P:tb, :], h2fb[:, :],
                    i2b_sb[:, cOff2b[b]: cOff2b[b] + szb // 16], szb, szb, ROW2,
                    single_packet=False, queue_num=1 + (2 * b + 1) % 3)
                seg2b = segs.tile([P, tb2max, P], bf16, tag="seg")
                nc.sync.dma_start(
                    out=seg2b[:, 0:tb, :],
                    in_=d_seg2[:, tOff2[b] * P: tOff2[b + 1] * P]
                    .rearrange("p (t q) -> p t q", q=P))
                segT2b = segs.tile([P, tb2max, P], bf16, tag="segT")
                nc.sync.dma_start(
                    out=segT2b[:, 0:tb, :],
                    in_=d_segT2[:, tOff2[b] * P: tOff2[b + 1] * P]
                    .rearrange("p (t q) -> p t q", q=P))

                pald2 = psA.tile([P, tb2max, 1], f32, tag="pald")
                for t in range(tb):
                    nc.tensor.matmul(out=pald2[:, t, :], lhsT=segT2b[:, t, :],
                                     rhs=ald2_sb[:, b, :], start=True, stop=True)
                pald2c = work.tile([P, tb2max, 1], bf16, tag="pald2c")
                nc.vector.tensor_copy(out=pald2c[:, 0:tb, :], in_=pald2[:, 0:tb, :])

                s2 = work.tile([P, tb2max, 1], f32, tag="s2")
                nc.vector.tensor_tensor(out=s2[:, 0:tb, :],
                                        in0=hg2[:, 0:tb, C:C + 2].bitcast(bf16),
                                        in1=pald2c[:, 0:tb, :], op=ALU.add)
                e2 = work.tile([P, tb2max, 1], f32, tag="e2")
                nc.vector.scalar_tensor_tensor(
                    out=e2[:, 0:tb, :], in0=s2[:, 0:tb, :], scalar=NEG_SLOPE,
                    in1=s2[:, 0:tb, :], op0=ALU.mult, op1=ALU.max)
                # exp lands straight in hgm column C, which doubles as the
                # softmax-denominator input to the aggregation matmul
                hgm = hgp2.tile([P, tb2max, C + 1], bf16, tag="hgm", bufs=2)
                nc.scalar.activation(hgm[:, 0:tb, C:C + 1], e2[:, 0:tb, :],
                                     AF.Exp)
                nc.vector.tensor_tensor(
                    out=hgm[:, 0:tb, 0:C], in0=hg2[:, 0:tb, 0:C],
                    in1=hgm[:, 0:tb, C:C + 1].to_broadcast([P, tb, C]),
                    op=ALU.mult)

                U2 = psU.tile([P, C + 1], f32, tag="U")
                for t in range(tb):
                    nc.tensor.matmul(out=U2[:, :], lhsT=seg2b[:, t, :],
                                     rhs=hgm[:, t, 0:C + 1],
                                     start=(t == 0), stop=(t == tb - 1))
                rec2 = work.tile([P, 1], f32, tag="rec2")
                nc.vector.reciprocal(rec2[0:nd, :], U2[0:nd, C:C + 1])
                o2r = work.tile([P, C], bf16, tag="o2r")
                nc.vector.scalar_tensor_tensor(
                    out=o2r[0:nd, :], in0=U2[0:nd, 0:C],
                    scalar=rec2[0:nd, 0:1], in1=b2b_sb[0:nd, :],
                    op0=ALU.mult, op1=ALU.add)
                nc.scalar.activation(o2r[0:nd, :], o2r[0:nd, :], AF.Relu)
                nc.tensor.matmul(out=poolT[:, :], lhsT=o2r[0:nd, :],
                                 rhs=poolm_sb[0:nd, b, :],
                                 start=(b == 0), stop=(b == nblk - 1))

            # ================= tail: pool exchange + classifier =============
            poolT_sb = work.tile([P, G], f32, tag="poolT_sb")
            nc.vector.tensor_copy(out=poolT_sb[:], in_=poolT[:, :])
            nc.sync.dma_start(out=pool_in[:, :], in_=poolT_sb[:])
            nc.gpsimd.collective_compute(
                "AllReduce", ALU.add,
                replica_groups=[list(range(n_cores))],
                ins=[pool_in.opt()], outs=[pool_out.opt()])
            poolF_sb = work.tile([P, G], f32, tag="poolF_sb")
            nc.sync.dma_start(out=poolF_sb[:], in_=pool_out[:, :])
            ofin = psD.tile([G, OUT], f32, tag="den")
            nc.tensor.matmul(out=ofin[:, :], lhsT=poolF_sb[:], rhs=wc_sb[:],
                             start=True, stop=True)
            ofin_sb = work.tile([G, OUT], f32, tag="ofin_sb")
            nc.vector.scalar_tensor_tensor(
                out=ofin_sb[:], in0=ofin[:, :], scalar=invc_sb[:, 0:1],
                in1=bcb_sb[:], op0=ALU.mult, op1=ALU.add)
            nc.sync.dma_start(out=d_out[:, :], in_=ofin_sb[:])

    nc.compile()
    return nc


# ------------------------------------------------------------------
#  runner
# ------------------------------------------------------------------

_CACHE = {}


def _get_nc(meta):
    key = (meta['n_cores'], meta['nblk'], meta['nloc'], meta['sz1'],
           meta['sz2'])
    if key not in _CACHE:
        _CACHE[key] = _build(meta)
    return _CACHE[key]


def _in_maps(common, per_core):
    maps = []
    for pc in per_core:
        m = dict(common)
        m.update(pc)
        maps.append(m)
    return maps


def kernel(**inputs) -> np.ndarray:
    common, per_core, meta = _prep(**inputs)
    nc = _get_nc(meta)
    from concourse.bass_utils import run_bass_kernel_spmd
    res = run_bass_kernel_spmd(nc, _in_maps(common, per_core),
                               core_ids=list(range(meta['n_cores'])))
    return np.asarray(res.results[0]['out'], np.float32).reshape(-1)
